# revision 13
# baseline (speedup 1.0000x reference)
"""NT-Xent loss kernel for Trainium2, 8-core SPMD.

Math: with p = cat(z_i, z_j) [8192, 64], pn = p / max(||p||, 1e-8),
sim = 2 * pn @ pn.T (TEMP=0.5), the reference's gather-based losses reduce to
  loss1 = mean_r( log(sum_{c != r} exp(sim[r,c])) - pos_r )
  loss2 = mean_r( log(exp(pos_r) + sum_{c != t_r} exp(probs[r,c])) - pos_r )
where pos_r = sim[r, (r+N) % 2N].  sim entries lie in [-2, 2], so the exp
never overflows and no max-shift pass is needed.  The huge neg_idx input is a
fixed structured mask (drop self + positive) and never needs to be read.

Sharding: row-parallel.  Each of the 8 cores receives ONLY its own 1024 rows
of p plus its probs / one-hot-target slices (one packed bf16 blob, ~172 KB);
it normalizes and transposes them locally, then an on-device AllGather of the
transposed bf16 blocks assembles the full [64, 8192] pnT every core needs for
the sim columns.  The positive-pair rows pn[(r+N)%2N] live on core k^4; a
second, pairwise AllGather over groups {k, k+4} brings them in, and pos is
recovered order-independently as
  pos = pns . (lo + hi) - diag        (lo+hi = own + partner rows)
so the same SPMD program works on every rank with static addressing.
Each core emits two partial sums; the host adds the 8 partials.

Wall-clock notes (the metric here is warm dispatch wall time; the axon
NTFF-profile path is unavailable in this container):
  - inputs shrank 21.7 MB -> 1.4 MB per call (host->device transfer over the
    axon tunnel was the dominant cost),
  - the jax persistent compilation cache makes repeat dispatches skip the
    client-side walrus NEFF recompile (~140 ms/call),
  - device work is minimized in instructions and elements: only the own
    shard is normalized/transposed per core, the one-hot target mask is
    precomputed on host, and the exp stream runs 32x [128, 2048] tiles with
    free ACT accum_out row-sums.
"""

import numpy as np

import jax

# Persistent compilation cache: run_bass_kernel_spmd builds a fresh jax.jit
# wrapper per call, so without this every dispatch re-runs the client-side
# walrus NEFF recompile. With it, identical HLO (same BIR) is a disk cache hit.
try:
    jax.config.update("jax_compilation_cache_dir", "/tmp/jax_comp_cache_ntx")
    jax.config.update("jax_persistent_cache_min_compile_time_secs", 0)
    jax.config.update("jax_persistent_cache_min_entry_size_bytes", -1)
except Exception:
    pass

import concourse.bass as bass
import concourse.bacc as bacc
import concourse.tile as tile
from concourse import mybir
from concourse.masks import make_identity
from concourse.bass_utils import run_bass_kernel_spmd

N = 4096
D = 64
M = 2 * N            # 8192 rows of sim
NCORES = 8
R = M // NCORES      # 1024 rows per core
NT = M // 128        # 64 row-tiles of the full p
NS = R // 128        # 8 row-tiles of a shard
NCLS = 10
INV_TEMP = 2.0       # 1 / 0.5
F32 = mybir.dt.float32
BF16 = mybir.dt.bfloat16

import os
# bf16 matmul for the sim slab: pos/diag stay fp32 (computed on DVE), and
# per-row errors average out over 8192 rows.
MM_W = int(os.environ.get("NTX_MMW", "512"))     # matmul free width
EXP_W = int(os.environ.get("NTX_EXPW", "2048"))  # exp tile width (PSUM)
NEWTON_ITERS = int(os.environ.get("NTX_NEWT", "2"))

AF = mybir.ActivationFunctionType
ALU = mybir.AluOpType


def _emit_rsqrt(nc, pool, n2, nchunk):
    """inv = 1/max(sqrt(n2), 1e-8), entirely on DVE: quake-style magic
    constant seed + Newton steps (ACT stays exclusively on Exp/Ln, so the
    activation table never thrashes)."""
    eng = nc.vector
    I32 = mybir.dt.int32
    inv = pool.tile([128, nchunk], F32, tag="rs_inv")
    eng.tensor_scalar(inv.bitcast(I32), n2.bitcast(I32), 1, None,
                      ALU.arith_shift_right)
    eng.tensor_scalar(inv.bitcast(I32), inv.bitcast(I32), -1, 0x5F3759DF,
                      ALU.mult, ALU.add)
    t2 = pool.tile([128, nchunk], F32, tag="rs_t2")
    for _ in range(NEWTON_ITERS):
        # y' = y * (1.5 - 0.5 * n2 * y^2)
        eng.tensor_mul(t2, inv, inv)
        eng.tensor_mul(t2, t2, n2)
        eng.tensor_scalar(t2, t2, -0.5, 1.5, ALU.mult, ALU.add)
        eng.tensor_mul(inv, inv, t2)
    eng.tensor_scalar_min(inv, inv, 1e8)
    return inv


def build_program():
    nc = bacc.Bacc("TRN2", target_bir_lowering=False, debug=False,
                   num_devices=NCORES)

    # One packed bf16 input per core: one h2d buffer per dispatch, half the
    # bytes of f32.  bf16 z costs ~0.2% per-element quantization that
    # averages out over the 8192-row loss means (validated ~1e-6 final).
    # Layout (bf16 elements):
    #   [0, R*D)                     zsh    — this core's rows of p
    #   [R*D, R*D+R*NCLS)            probs  — this core's probs rows
    #   [R*D+R*NCLS, R*D+2*R*NCLS)   onehot — (c == target_r) as 1.0/0.0
    BLOB = R * D + 2 * R * NCLS
    blob_d = nc.dram_tensor("blob", [1, BLOB], BF16,
                            kind="ExternalInput").ap()
    zsh_d = blob_d[0, 0:R * D].rearrange("(n p d) -> p n d", p=128, d=D)
    probs_d = blob_d[0, R * D:R * D + R * NCLS].rearrange(
        "(n p c) -> p n c", p=128, c=NCLS)
    oneh_d = blob_d[0, R * D + R * NCLS:BLOB].rearrange(
        "(n p c) -> p n c", p=128, c=NCLS)
    out_d = nc.dram_tensor("out", [1, 2], F32, kind="ExternalOutput").ap()

    JJ = M // EXP_W       # exp tiles per shard row-tile
    NMM = EXP_W // MM_W   # matmuls per exp tile

    with tile.TileContext(nc) as tc:
        import contextlib
        with contextlib.ExitStack() as ctx:
            consts = ctx.enter_context(tc.tile_pool(name="consts", bufs=1))
            big = ctx.enter_context(tc.tile_pool(name="big", bufs=1))
            work = ctx.enter_context(tc.tile_pool(name="work", bufs=2))
            tp = ctx.enter_context(
                tc.tile_pool(name="tp", bufs=3, space="PSUM"))
            mm = ctx.enter_context(
                tc.tile_pool(name="mm", bufs=1, space="PSUM"))
            po = ctx.enter_context(
                tc.tile_pool(name="po", bufs=1, space="PSUM"))
            dr = ctx.enter_context(
                tc.tile_pool(name="dr", bufs=1, space="DRAM"))

            identity = consts.tile([128, 128], BF16)
            make_identity(nc, identity)
            ones = consts.tile([128, 1], F32)
            nc.vector.memset(ones, 1.0)

            # ---- load + normalize this core's shard (fp32, DVE) ----
            rawsq_b = big.tile([128, NS, D], BF16)
            nc.sync.dma_start(out=rawsq_b, in_=zsh_d)
            rawsq = big.tile([128, NS, D], F32)
            nc.vector.tensor_copy(rawsq.rearrange("p n d -> p (n d)"),
                                  rawsq_b.rearrange("p n d -> p (n d)"))

            sflat = rawsq.rearrange("p n d -> p (n d)")
            s_sq = big.tile([128, NS * D], F32)
            s_n2 = big.tile([128, NS], F32)
            nc.vector.tensor_mul(s_sq, sflat, sflat)
            nc.vector.tensor_reduce(
                s_n2, s_sq.rearrange("p (n d) -> p n d", d=D),
                axis=mybir.AxisListType.X, op=ALU.add)
            s_inv = _emit_rsqrt(nc, big, s_n2, NS)
            pns = big.tile([128, NS, D], F32)
            for n in range(NS):
                nc.vector.tensor_scalar_mul(pns[:, n, :], rawsq[:, n, :],
                                            s_inv[:, n:n + 1])

            # bf16 copy + transposes -> psT [64, R] (lhsT for the slab, and
            # this core's contribution to the transposed all-gather)
            pnsb = big.tile([128, NS, D], BF16)
            nc.vector.tensor_copy(
                pnsb.rearrange("p n d -> p (n d)"),
                pns.rearrange("p n d -> p (n d)"))
            psT = big.tile([64, R], BF16)
            for q4 in range(NS // 4):
                tpp = tp.tile([64, 512], BF16, tag="tp")
                for q in range(4):
                    nn = 4 * q4 + q
                    nc.tensor.transpose(
                        tpp[:, q * 128:(q + 1) * 128], pnsb[:, nn, :],
                        identity)
                nc.vector.tensor_copy(psT[:, q4 * 512:(q4 + 1) * 512], tpp)

            # ---- on-device gathers ----
            # (1) transposed bf16 blocks -> pnT [64, 8192] via 8 block DMAs
            aginT = dr.tile([64, R], BF16)
            nc.sync.dma_start(out=aginT, in_=psT)
            agT = dr.tile([NCORES * 64, R], BF16, addr_space="Shared")
            nc.gpsimd.collective_compute(
                "AllGather", ALU.bypass,
                replica_groups=[list(range(NCORES))],
                ins=[aginT], outs=[agT])
            # (2) fp32 normalized rows of the pair {k, k+4} for pos
            agin = dr.tile([R, D], F32)
            nc.sync.dma_start(
                out=agin.rearrange("(n p) d -> p n d", p=128), in_=pns)
            agpair = dr.tile([2 * R, D], F32)
            nc.gpsimd.collective_compute(
                "AllGather", ALU.bypass,
                replica_groups=[[k, k + 4] for k in range(4)],
                ins=[agin], outs=[agpair])

            pnT = big.tile([64, M], BF16)
            for r in range(NCORES):
                nc.sync.dma_start(out=pnT[:, r * R:(r + 1) * R],
                                  in_=agT[r * 64:(r + 1) * 64, :])

            # probs part: exp on ACT, one-hot own-class sum from host mask
            probs_b = big.tile([128, NS, NCLS], BF16)
            nc.sync.dma_start(out=probs_b, in_=probs_d)
            probs_t = big.tile([128, NS, NCLS], F32)
            nc.vector.tensor_copy(probs_t.rearrange("p n c -> p (n c)"),
                                  probs_b.rearrange("p n c -> p (n c)"))
            oneh_b = big.tile([128, NS, NCLS], BF16)
            nc.sync.dma_start(out=oneh_b, in_=oneh_d)
            eprobs = big.tile([128, NS, NCLS], F32)
            nc.scalar.activation(
                eprobs.rearrange("p n c -> p (n c)"),
                probs_t.rearrange("p n c -> p (n c)"), AF.Exp)

            # ---- the sim slab: 8 row-tiles x JJ exp tiles of [128, EXP_W],
            # row sums via free ACT accum_out ----
            scols = big.tile([128, NS * JJ], F32)
            for n in range(NS):
                lhsT = psT[:, n * 128:(n + 1) * 128]
                for jj in range(JJ):
                    idx = n * JJ + jj
                    c0 = jj * EXP_W
                    pst = mm.tile([128, EXP_W], F32, tag="mm")
                    for q in range(NMM):
                        nc.tensor.matmul(
                            pst[:, q * MM_W:(q + 1) * MM_W], lhsT,
                            pnT[:, c0 + q * MM_W:c0 + (q + 1) * MM_W],
                            start=True, stop=True)
                    # in-place exp on the PSUM tile: the elementwise output
                    # is never read (only accum_out is), and ScE->PSUM is
                    # the faster ACT destination
                    nc.scalar.activation(
                        pst, pst, AF.Exp, scale=INV_TEMP,
                        accum_out=scols[:, idx:idx + 1])

            own = big.tile([128, NS], F32)
            omul = work.tile([128, NS, NCLS], F32, tag="omul")
            nc.vector.tensor_mul(
                omul.rearrange("p n c -> p (n c)"),
                eprobs.rearrange("p n c -> p (n c)"),
                oneh_b.rearrange("p n c -> p (n c)"))
            nc.vector.tensor_reduce(own, omul, axis=mybir.AxisListType.X,
                                    op=ALU.add)
            sum10 = big.tile([128, NS], F32)
            nc.vector.tensor_reduce(sum10, eprobs, axis=mybir.AxisListType.X,
                                    op=ALU.add)

            # pos_r and diag_r in fp32 — pair-gathered rows give
            # lo+hi = own + partner, so pos = pns.(lo+hi) - diag with static
            # addressing on every rank.  Emitted late so their DVE/ACT ops
            # cannot stall the main exp stream.
            prl = big.tile([128, NS, D], F32)
            nc.sync.dma_start(
                out=prl,
                in_=agpair[0:R, :].rearrange("(n p) d -> p n d", p=128))
            prh = big.tile([128, NS, D], F32)
            nc.sync.dma_start(
                out=prh,
                in_=agpair[R:2 * R, :].rearrange("(n p) d -> p n d", p=128))
            psum_rows = work.tile([128, NS, D], F32, tag="rowdot", bufs=2)
            nc.vector.tensor_add(psum_rows, prl, prh)

            diag_raw = big.tile([128, NS], F32)
            dsum_raw = big.tile([128, NS], F32)
            dq = work.tile([128, NS, D], F32, tag="rowdot", bufs=2)
            nc.vector.tensor_mul(dq, pns, pns)
            nc.vector.tensor_reduce(diag_raw, dq, axis=mybir.AxisListType.X,
                                    op=ALU.add)
            pq = work.tile([128, NS, D], F32, tag="rowdot", bufs=2)
            nc.vector.tensor_mul(pq, pns, psum_rows)
            nc.vector.tensor_reduce(dsum_raw, pq, axis=mybir.AxisListType.X,
                                    op=ALU.add)
            pos_raw = big.tile([128, NS], F32)
            nc.vector.tensor_sub(pos_raw, dsum_raw, diag_raw)

            ediag = big.tile([128, NS], F32)
            nc.scalar.activation(ediag, diag_raw, AF.Exp, scale=INV_TEMP)
            epos = big.tile([128, NS], F32)
            nc.scalar.activation(epos, pos_raw, AF.Exp, scale=INV_TEMP)
            pos2 = big.tile([128, NS], F32)
            nc.vector.tensor_scalar_mul(pos2, pos_raw, INV_TEMP)

            # ---- loss tails ----
            stot = big.tile([128, NS], F32)
            nc.vector.tensor_reduce(
                stot, scols.rearrange("p (n j) -> p n j", j=JJ),
                axis=mybir.AxisListType.X, op=ALU.add)
            s1 = big.tile([128, NS], F32)
            nc.vector.tensor_sub(s1, stot, ediag)
            lse1 = big.tile([128, NS], F32)
            nc.scalar.activation(lse1, s1, AF.Ln)
            c1 = big.tile([128, NS], F32)
            nc.vector.tensor_sub(c1, lse1, pos2)
            v12 = big.tile([128, 2], F32)
            nc.vector.tensor_reduce(v12[:, 0:1], c1,
                                    axis=mybir.AxisListType.X, op=ALU.add)

            s2 = big.tile([128, NS], F32)
            nc.vector.tensor_sub(s2, sum10, own)
            nc.vector.tensor_add(s2, s2, epos)
            # false data-dep on stot so the scheduler cannot hoist the Ln
            # into the exp stream (each hoist costs 2 ACT table swaps)
            nc.vector.scalar_tensor_tensor(
                out=s2, in0=stot, scalar=0.0, in1=s2,
                op0=ALU.mult, op1=ALU.add)
            lse2 = big.tile([128, NS], F32)
            nc.scalar.activation(lse2, s2, AF.Ln)
            c2 = big.tile([128, NS], F32)
            nc.vector.tensor_sub(c2, lse2, pos2)
            nc.vector.tensor_reduce(v12[:, 1:2], c2,
                                    axis=mybir.AxisListType.X, op=ALU.add)

            # ---- partition-sum via ones-matmul, then DMA out ----
            pso = po.tile([1, 2], F32)
            nc.tensor.matmul(pso, ones, v12, start=True, stop=True)
            outsb = big.tile([1, 2], F32)
            nc.vector.tensor_copy(outsb, pso)
            nc.sync.dma_start(out=out_d, in_=outsb)

    nc.compile()
    return nc


_NC_CACHE = None


def _get_nc():
    global _NC_CACHE
    if _NC_CACHE is None:
        _NC_CACHE = build_program()
    return _NC_CACHE


def make_in_maps(z_i, z_j, probs, target):
    import ml_dtypes
    p = np.ascontiguousarray(
        np.concatenate([z_i, z_j], axis=0), dtype=np.float32)
    t2 = np.concatenate([target, target]).astype(np.int64)
    probs = np.asarray(probs, dtype=np.float32)
    onehot = (np.arange(NCLS)[None, :] == t2[:, None]).astype(np.float32)
    in_maps = []
    for k in range(NCORES):
        lo = k * R
        blob = np.concatenate([
            p[lo:lo + R].reshape(-1),
            probs[lo:lo + R].reshape(-1),
            onehot[lo:lo + R].reshape(-1),
        ]).reshape(1, -1).astype(ml_dtypes.bfloat16)
        in_maps.append({"blob": np.ascontiguousarray(blob)})
    return in_maps


def kernel(z_i, z_j, probs, target, neg_idx):
    # neg_idx is the fixed structured NT-Xent mask (all columns except self and
    # positive); its effect is computed analytically, so it is never read.
    del neg_idx
    nc = _get_nc()
    in_maps = make_in_maps(np.asarray(z_i), np.asarray(z_j),
                           np.asarray(probs), np.asarray(target))
    res = run_bass_kernel_spmd(nc, in_maps, list(range(NCORES)))
    parts = np.stack([res.results[k]["out"].reshape(2) for k in range(NCORES)])
    total = parts.sum(axis=0) / np.float32(M)
    l1 = np.float32(total[0])
    l2 = np.float32(total[1])
    return (np.asarray(l1), np.asarray(l2))
